# revision 1
# baseline (speedup 1.0000x reference)
"""Trainium2 Bass kernel for nn_Attention_86646670230179 (eager MHA, f32 I/O).

Strategy (8 NeuronCores, tensor-parallel over heads, collective-free):
  - Each core owns 2 of the 16 heads (a 128-row slice of the internal dim).
  - Host stages q/k/v as [128, B, 2, KT, 1024] bf16 (each L-half contiguous
    per partition: long DMA lines, ~10 issue instructions total -- issue
    time on the queues, not bandwidth, gated the first exp). First-half
    columns land first so the first attention block runs while the second
    half streams (its pair-1 projections ride inside that block as per-st
    fillers). Score scale (1/8) folded into Wq.
  - Per core: qp^T/kp^T projections (transposed layout), vp projection
    (natural layout via on-chip PE transpose), scores^T = kh^T.T @ qh^T with
    both heads packed into the 128-row PE array, exp on ScalarE (no max
    subtraction: scores ~ N(0,1)), PV matmul with an appended ones-column
    producing unnormalized outputs + row sums in one PSUM accumulation.
    Normalization runs immediately after each block's PV stop:
    reciprocal_approx_fast on the [1,512] sums row (staged to SBUF first —
    the custom DVE op cannot read PSUM), gpsimd partition-broadcast, one
    vector multiply per head.
  - Each core then applies its slice of the output projection (deferred one
    block, spread one matmul per st iteration so a vector dependency never
    stalls the in-order PE queue) and writes bf16 partial outputs; the host
    sums the 8 partials (the all-reduce of the TP sharding) and adds
    (bv @ Wo + bo), which commutes with attention because softmax rows sum
    to 1.
  - fp8 was evaluated and rejected: DoubleRow fp8 PV is throughput-neutral
    here (the part DVFS-throttles under the extra PE activity, slowing every
    engine ~18%), and fp8 staging of q/k/v costs ~3% output error (value-
    path quantization does not average out), over the 2e-2 gate.
"""
import sys
from contextlib import ExitStack

import numpy as np

sys.path.insert(0, "/opt/trn_rl_repo")

import ml_dtypes  # noqa: E402
import concourse.bass as bass  # noqa: E402
import concourse.mybir as mybir  # noqa: E402
import concourse.tile as tile  # noqa: E402
from concourse import bacc  # noqa: E402
from concourse.bass_utils import run_bass_kernel_spmd  # noqa: E402
from concourse.masks import make_identity  # noqa: E402

BF16 = mybir.dt.bfloat16
F32 = mybir.dt.float32
F8 = mybir.dt.float8e4
AF = mybir.ActivationFunctionType
DROW = mybir.MatmulPerfMode.DoubleRow

NCORES = 8
B, L, E, H = 2, 2048, 1024, 16
S = L
D = E // H            # 64 head dim
R = B * L             # 4096 total rows
HC = H // NCORES      # 2 heads per core
EC = HC * D           # 128 channel slice per core
KT = E // 128         # 8 contraction tiles
NT = L // 512         # 4 512-wide row tiles per batch
NP = L // 1024        # 2 1024-wide projection pairs per batch
ST = S // 128         # 16 key tiles per batch
STN = ST // NT        # 4 key tiles per 512-row block
DP1 = D + 1           # 65: head dim + ones column
DP2 = D + 8           # 72: + zero pad; dual-fp8 LDWEIGHTS needs the
                      # pair-plane step (2*DP2 fp8 bytes) 16B-aligned


def build_nc():
    nc = bacc.Bacc("TRN2", target_bir_lowering=False, num_devices=NCORES)

    qT = nc.declare_dram_parameter("qT", [128, B, 2, KT, 1024], BF16,
                                   isOutput=False)
    kT = nc.declare_dram_parameter("kT", [128, B, 2, KT, 1024], BF16,
                                   isOutput=False)
    vT = nc.declare_dram_parameter("vT", [128, B, 2, KT, 1024], BF16,
                                   isOutput=False)
    wq = nc.declare_dram_parameter("wq", [128, KT * EC], BF16, isOutput=False)
    wk = nc.declare_dram_parameter("wk", [128, KT * EC], BF16, isOutput=False)
    wv = nc.declare_dram_parameter("wv", [128, KT * EC], BF16, isOutput=False)
    wo = nc.declare_dram_parameter("wo", [128, E], BF16, isOutput=False)
    bq = nc.declare_dram_parameter("bq", [EC, 1], F32, isOutput=False)
    bk = nc.declare_dram_parameter("bk", [EC, 1], F32, isOutput=False)
    outTp = nc.declare_dram_parameter("outTp", [E, R], BF16, isOutput=True)

    with tile.TileContext(nc) as tc, ExitStack() as ctx:
        consts = ctx.enter_context(tc.tile_pool(name="consts", bufs=1))
        xt_pool = ctx.enter_context(tc.tile_pool(name="xt", bufs=1))
        vpt_pool = ctx.enter_context(tc.tile_pool(name="vpt", bufs=2))
        exp_pool = ctx.enter_context(tc.tile_pool(name="expp", bufs=4))
        ot_pool = ctx.enter_context(tc.tile_pool(name="otp", bufs=2))
        ov_pool = ctx.enter_context(tc.tile_pool(name="ovp", bufs=3))
        rc_pool = ctx.enter_context(tc.tile_pool(name="rcp", bufs=3))
        rcb_pool = ctx.enter_context(tc.tile_pool(name="rcbp", bufs=3))
        # PSUM banks: sc 2x[128,1024] (4) + pv 3x[128,512] (3) + pp 1x[128,512]
        psum_sc = ctx.enter_context(tc.tile_pool(name="psc", bufs=2, space="PSUM"))
        psum_pv = ctx.enter_context(tc.tile_pool(name="ppv", bufs=3, space="PSUM"))
        psum_pp = ctx.enter_context(tc.tile_pool(name="ppp", bufs=1, space="PSUM"))

        # ---- weights staging first (small; PE needs them immediately)
        wq_sb = consts.tile([128, KT, EC], BF16, tag="wq")
        wk_sb = consts.tile([128, KT, EC], BF16, tag="wk")
        wv_sb = consts.tile([128, KT, EC], BF16, tag="wv")
        wo_sb = consts.tile([128, KT, EC], BF16, tag="wo")
        for w_sb, w in ((wk_sb, wk), (wv_sb, wv), (wq_sb, wq)):
            nc.sync.dma_start(w_sb[:], w[:].rearrange("p (ko m) -> p ko m", m=EC))
        nc.sync.dma_start(wo_sb[:], wo[:].rearrange("p (m o) -> p m o", o=EC))
        bq_sb = consts.tile([EC, 1], F32, tag="bq")
        bk_sb = consts.tile([EC, 1], F32, tag="bk")
        nc.gpsimd.dma_start(bq_sb[:], bq[:])
        nc.gpsimd.dma_start(bk_sb[:], bk[:])
        ident = consts.tile([128, 128], BF16, tag="ident")
        make_identity(nc, ident[:])

        # projected activations: 1024-wide pair tiles
        qpT = [[consts.tile([128, 1024], BF16, tag=f"qpT{b}_{p}",
                            name=f"qpT{b}_{p}") for p in range(NP)]
               for b in range(B)]
        kpT = [[consts.tile([128, 1024], BF16, tag=f"kpT{b}_{p}",
                            name=f"kpT{b}_{p}") for p in range(NP)]
               for b in range(B)]
        vp = [[consts.tile([128, STN, 2 * DP1], BF16, tag=f"vp{b}_{n}",
                           name=f"vp{b}_{n}")
               for n in range(NT)] for b in range(B)]
        for b in range(B):
            for n in range(NT):
                nc.vector.memset(vp[b][n][:, :, D], 1.0)
                nc.vector.memset(vp[b][n][:, :, 2 * D + 1], 1.0)

        def stage0():
            """Batch-0 staging on the scalar queue. The host layout keeps
            each L-half contiguous per partition (16KB lines), so the whole
            batch needs only 10 issue instructions -- issue time on the
            scalar queue, not bandwidth, gated the first exp. k is chunked
            for projection pacing; v/q land as one blob per half."""
            staged = {}
            for name, xsrc in (("k", kT), ("v", vT), ("q", qT)):
                staged[name] = xt_pool.tile([128, KT, L], BF16,
                                            tag=f"xt{name}", name=f"xt{name}0")
            for j in range(4):
                nc.scalar.dma_start(staged["k"][:, 2 * j:2 * j + 2, 0:1024],
                                    kT[:, 0, 0, 2 * j:2 * j + 2, :])
            nc.scalar.dma_start(staged["v"][:, :, 0:1024], vT[:, 0, 0])
            nc.scalar.dma_start(staged["q"][:, :, 0:1024], qT[:, 0, 0])
            for j in range(2):
                nc.scalar.dma_start(staged["k"][:, 4 * j:4 * j + 4, 1024:2048],
                                    kT[:, 0, 1, 4 * j:4 * j + 4, :])
            nc.scalar.dma_start(staged["v"][:, :, 1024:2048], vT[:, 0, 1])
            nc.scalar.dma_start(staged["q"][:, :, 1024:2048], qT[:, 0, 1])
            return staged

        def stage1(names):
            """Batch-1 staging on gpsimd, one blob per half per tensor (2
            issues each) so the issue queue never delays the normalization
            broadcasts that share it."""
            staged = {}
            for name, xsrc in (("k", kT), ("v", vT), ("q", qT)):
                if name not in names:
                    continue
                xt = xt_pool.tile([128, KT, L], BF16, tag=f"xt{name}",
                                  name=f"xt{name}1")
                for h in range(2):
                    nc.gpsimd.dma_start(xt[:, :, h * 1024:(h + 1) * 1024],
                                        xsrc[:, 1, h])
                staged[name] = xt
            return staged

        def project(b, staged, pairs=None):
            """kp^T, vp (natural), qp^T for batch b, 1024-column pair tiles."""
            for name, w_sb, bias in (
                ("k", wk_sb, bk_sb),
                ("v", wv_sb, None),
                ("q", wq_sb, bq_sb),
            ):
                xts = staged[name]
                for p in (pairs if pairs is not None else range(NP)):
                    ps = psum_sc.tile([128, 1024], F32, tag="sc", name="psp")
                    for kt in range(KT):
                        for hf in range(2):  # matmul out is capped at one
                            nc.tensor.matmul(  # PSUM bank (512 f32 columns)
                                ps[:, hf * 512:(hf + 1) * 512],
                                lhsT=w_sb[:, kt, :],
                                rhs=xts[:, kt,
                                        p * 1024 + hf * 512:
                                        p * 1024 + (hf + 1) * 512],
                                start=(kt == 0),
                                stop=(kt == KT - 1),
                            )
                    if name == "k":
                        nc.vector.tensor_tensor(
                            kpT[b][p][:], ps[:],
                            bias[:].to_broadcast((EC, 1024)), mybir.AluOpType.add,
                        )
                    elif name == "q":
                        nc.vector.tensor_tensor(
                            qpT[b][p][:], ps[:],
                            bias[:].to_broadcast((EC, 1024)), mybir.AluOpType.add,
                        )
                    else:
                        vpt = vpt_pool.tile([128, 1024], BF16, tag="vpt")
                        nc.vector.tensor_copy(vpt[:], ps[:])
                        for mb in range(2 * STN):
                            n = p * 2 + mb // STN
                            sl = mb % STN
                            trp = psum_pp.tile([128, 128], BF16, tag="pp")
                            nc.tensor.transpose(
                                trp[:], vpt[:, mb * 128:(mb + 1) * 128], ident[:]
                            )
                            nc.vector.tensor_copy(
                                vp[b][n][:, sl, 0:D], trp[:, 0:D]
                            )
                            nc.vector.tensor_copy(
                                vp[b][n][:, sl, DP1:DP1 + D], trp[:, D:2 * D]
                            )
                    if name != "v":
                        proj_one()

        pending_proj = []  # deferred out-projection tiles: (ot, rowbase, m)

        def proj_one(pool=None, cast_eng=None, dma_eng=None):
            # one 128-col partial out-projection tile (spread across steps)
            if not pending_proj:
                return
            ot, rowbase, m = pending_proj.pop(0)
            pl = pool if pool is not None else psum_pp
            tg = "pv" if pl is psum_pv else ("pp" if pl is psum_pp else "sc")
            pt = pl.tile([128, 512], F32, tag=tg, name="ptp")
            nc.tensor.matmul(
                pt[:], lhsT=wo_sb[:, m, :], rhs=ot[:],
                start=True, stop=True,
            )
            ov = ov_pool.tile([128, 512], BF16, tag="ov")
            if cast_eng is nc.scalar:
                nc.scalar.copy(ov[:], pt[:])
            else:
                nc.vector.tensor_copy(ov[:], pt[:])
            (dma_eng or nc.sync).dma_start(
                outTp[m * 128:(m + 1) * 128, rowbase:rowbase + 512], ov[:]
            )

        def attention(b, lt, filler=None, pre_pv=None):
            """One 512-row query block: both heads, full softmax + PV.

            Normalization runs immediately after the PV accumulation stops;
            the out-projection is deferred into the NEXT block's st loop
            (one matmul per step) so a vector-engine dependency never stalls
            the in-order PE queue. `filler`/`pre_pv` hooks let the first
            block interleave pair-1 projection work while its staging DMA
            is still landing.
            """
            po = []
            for h in range(HC):
                p = psum_pv.tile([128, 512], F32, tag="pv", name=f"po{h}")
                po.append(p)
            pp = lt // 2
            for st in range(ST):
                ps = psum_sc.tile([128, 1024], F32, tag="sc", name="psc")
                for h in range(HC):
                    nc.tensor.matmul(
                        ps[:, h * 512:(h + 1) * 512],
                        lhsT=kpT[b][st // 8][h * D:(h + 1) * D,
                                             (st % 8) * 128:(st % 8 + 1) * 128],
                        rhs=qpT[b][pp][h * D:(h + 1) * D,
                                       (lt % 2) * 512:(lt % 2) * 512 + 512],
                        start=True,
                        stop=True,
                        tile_position=(h * D, 0),
                    )
                ex = exp_pool.tile([128, 1024], BF16, tag="exp")
                nc.scalar.activation(ex[:], ps[:], AF.Exp)
                if pre_pv is not None:
                    pre_pv(st)
                for h in range(HC):
                    nc.tensor.matmul(
                        po[h][0:DP1, :],
                        lhsT=vp[b][st // STN][:, st % STN, h * DP1:(h + 1) * DP1],
                        rhs=ex[:, h * 512:(h + 1) * 512],
                        start=(st == 0),
                        stop=(st == ST - 1),
                    )
                if filler is not None:
                    filler(st)
                elif st >= 2 and (st % 2 == 0 or st == 15):
                    # every other st: keeps the PE loaded through sts 10-15
                    # where it otherwise drains ahead of the scalar exp
                    proj_one()
            # ---- immediate normalization (cheap chain, off the PE queue)
            ot = ot_pool.tile([128, 512], BF16, tag="ot")
            rcbs = []
            for h in range(HC):
                sm = rc_pool.tile([1, 512], F32, tag="sm")
                nc.vector.tensor_copy(sm[:], po[h][D:DP1, :])
                rc1 = rc_pool.tile([1, 512], F32, tag="rc1")
                nc.vector.reciprocal_approx_fast(rc1[:], sm[:])
                rcb = rcb_pool.tile([D, 512], F32, tag="rcb")
                nc.gpsimd.partition_broadcast(rcb[:], rc1[:])
                rcbs.append(rcb)
            # muls AFTER both recip chains: h0's broadcast overlaps h1's
            # copy+recip instead of blocking it on the in-order vector queue
            for h in range(HC):
                nc.vector.tensor_tensor(
                    ot[h * D:(h + 1) * D, :], po[h][0:D, :], rcbs[h][:],
                    mybir.AluOpType.mult,
                )
            rowbase = b * L + lt * 512
            for m in range(KT):
                pending_proj.append((ot, rowbase, m))

        staged0 = stage0()
        project(0, staged0, pairs=[0])

        def make_chain_ops(name, w_sb, bias):
            ops = []
            box = {}
            for hf in range(2):
                for kt in range(KT):
                    def mm(kt=kt, hf=hf, name=name, w_sb=w_sb):
                        if kt == 0:
                            box[hf] = psum_pp.tile([128, 512], F32, tag="pp",
                                                   name="strp")
                        nc.tensor.matmul(
                            box[hf][:], lhsT=w_sb[:, kt, :],
                            rhs=staged0[name][:, kt,
                                              1024 + hf * 512:1536 + hf * 512],
                            start=(kt == 0), stop=(kt == KT - 1),
                        )
                    ops.append(mm)

                def epi(hf=hf, name=name, bias=bias):
                    dst = (kpT if name == "k" else qpT)[0][1]
                    nc.vector.tensor_tensor(
                        dst[:, hf * 512:(hf + 1) * 512], box[hf][:],
                        bias[:].to_broadcast((EC, 512)), mybir.AluOpType.add,
                    )
                ops.append(epi)
            return ops

        def emit_v_pair1():
            for hf in range(2):
                ps = psum_pp.tile([128, 512], F32, tag="pp", name="vps")
                for kt in range(KT):
                    nc.tensor.matmul(
                        ps[:], lhsT=wv_sb[:, kt, :],
                        rhs=staged0["v"][:, kt,
                                         1024 + hf * 512:1536 + hf * 512],
                        start=(kt == 0), stop=(kt == KT - 1),
                    )
                vpt = vpt_pool.tile([128, 512], BF16, tag="vpt", name="vpth")
                nc.vector.tensor_copy(vpt[:], ps[:])
                for sl in range(STN):
                    trp = psum_pp.tile([128, 128], BF16, tag="pp", name="trps")
                    nc.tensor.transpose(
                        trp[:], vpt[:, sl * 128:(sl + 1) * 128], ident[:]
                    )
                    nc.vector.tensor_copy(vp[0][2 + hf][:, sl, 0:D],
                                          trp[:, 0:D])
                    nc.vector.tensor_copy(vp[0][2 + hf][:, sl, DP1:DP1 + D],
                                          trp[:, D:2 * D])

        kops = make_chain_ops("k", wk_sb, bk_sb)
        qops = make_chain_ops("q", wq_sb, bq_sb)
        kpops = [3, 3, 2, 2, 2, 2, 2, 2]

        def stream_filler(st):
            if st < 8:
                for _ in range(kpops[st]):
                    if kops:
                        kops.pop(0)()
            elif st >= 10:
                for _ in range(3):
                    if qops:
                        qops.pop(0)()

        def stream_pre_pv(st):
            if st == 8:
                emit_v_pair1()

        attention(0, 0, filler=stream_filler, pre_pv=stream_pre_pv)
        staged1 = {}
        staged1.update(stage1(("k", "v")))
        attention(0, 1)
        staged1.update(stage1(("q",)))
        attention(0, 2)
        attention(0, 3)
        project(1, staged1)
        for lt in range(NT):
            attention(1, lt)
        # final flush: rotate four psum slots, split casts between scalar and
        # vector, and split DMA issues between sync and gpsimd so the tail
        # pipelines instead of serializing on any one engine
        i = 0
        while pending_proj:
            proj_one(pool=(psum_sc if i % 3 < 2 else psum_pp))
            i += 1

    nc.compile()
    return nc


_NC_CACHE = {}


def _get_nc():
    if "nc" not in _NC_CACHE:
        _NC_CACHE["nc"] = build_nc()
    return _NC_CACHE["nc"]


def _prearrange(w):
    # [E, EC] -> [128, KT*EC] partition-major so the device DMA is contiguous
    bf = ml_dtypes.bfloat16
    return np.ascontiguousarray(
        w.reshape(KT, 128, EC).transpose(1, 0, 2).reshape(128, KT * EC)
    ).astype(bf)


def kernel(q, k, v, Wq, bq, Wk, bk, Wv, bv, Wo, bo, _trace=False, _tmpdir=None):
    bf = ml_dtypes.bfloat16
    scale = np.float32(1.0 / np.sqrt(D))  # 0.125, exact

    def _stage_x(x):
        # [B, L, E] -> [128, B, 2, KT, 1024]: partition-major staging with
        # each L-half contiguous per partition (long DMA lines, few issues)
        xt = np.asarray(x, np.float32).reshape(B, 2, 1024, KT, 128)
        return np.ascontiguousarray(xt.transpose(4, 0, 1, 3, 2)).astype(bf)

    qTh = _stage_x(q)
    kTh = _stage_x(k)
    vTh = _stage_x(v)
    Wq = np.asarray(Wq, np.float32)
    Wk = np.asarray(Wk, np.float32)
    Wv = np.asarray(Wv, np.float32)
    Wo = np.asarray(Wo, np.float32)

    in_maps = []
    for c in range(NCORES):
        sl = slice(c * EC, (c + 1) * EC)
        in_maps.append({
            "qT": qTh,
            "kT": kTh,
            "vT": vTh,
            "wq": _prearrange(Wq[:, sl] * scale),
            "wk": _prearrange(Wk[:, sl]),
            "wv": _prearrange(Wv[:, sl]),
            "wo": np.ascontiguousarray(Wo[sl, :]).astype(bf),
            "bq": (np.asarray(bq, np.float32)[sl] * scale).reshape(EC, 1).copy(),
            "bk": np.asarray(bk, np.float32)[sl].reshape(EC, 1).copy(),
        })

    nc = _get_nc()
    res = run_bass_kernel_spmd(
        nc, in_maps, list(range(NCORES)), trace=_trace, tmpdir=_tmpdir
    )
    # sum the per-core partial outputs (the all-reduce of the TP sharding)
    acc = np.zeros((E, R), np.float32)
    for c in range(NCORES):
        acc += np.asarray(res.results[c]["outTp"], np.float32)
    out = np.ascontiguousarray(acc.T)  # [R, E]
    # bv passes through attention unchanged (softmax rows sum to 1):
    # out += bv @ Wo + bo
    host_bias = (
        np.asarray(bv, np.float64) @ np.asarray(Wo, np.float64)
        + np.asarray(bo, np.float64)
    ).astype(np.float32)
    out += host_bias[None, :]
    if _trace:
        return out.reshape(B, L, E), res
    return out.reshape(B, L, E)



# revision 7
# speedup vs baseline: 1.0808x; 1.0808x over previous
"""Trainium2 Bass kernel for nn_Attention_86646670230179 (eager MHA, f32 I/O).

Strategy (8 NeuronCores, tensor-parallel over heads, collective-free):
  - Each core owns 2 of the 16 heads (a 128-row slice of the internal dim).
  - The scalar engine is the critical resource (128 exp ACTIVATEs of
    [128,1024], ~1.11us each = 143us).  This schedule keeps it doing ONLY
    exp: staging DMA issues live on the gpsimd ring, out-proj casts on
    vector, and every projection (both batches) plus the out-projection
    runs as PE "filler" work threaded between the scores/PV matmuls of the
    scalar-bound st loop.
  - Staging DMAs are 512-row chunks in need-order (k of batch 0 first,
    v/q interleaved by first-use) so block (0,0) is DMA-gated as little
    as possible.  Host stages q/k/v as [128, B, 2, KT, 1024] bf16.
  - Per st: scores^T for both heads as a tile_position-packed concurrent
    pair into one f32 PSUM tile; exp on ScalarE (no max subtraction:
    scores ~ N(0,1), scale 1/8 folded into Wq); PV with an appended
    ones-column (unnormalized out + row sums in one accumulation).
  - Normalization: one PSUM->SBUF copy per head (releases the PV bank
    within ~0.7us so 2 PV banks suffice), reciprocal_approx_fast, gpsimd
    partition-broadcast, one vector multiply per head.
  - PSUM banks: scores 2x[128,1024] (4) + PV 2x[128,512] (2) + proj chain
    (1) + out-proj (1) = 8.
  - Host sums the 8 bf16 partial outputs (the TP all-reduce) and adds
    (bv @ Wo + bo), which commutes with attention since softmax rows sum
    to 1.  fp8 was evaluated and rejected: attention-weight quantization
    gives ~3-5% output error, over the 2e-2 gate.
"""
import sys
from contextlib import ExitStack

import numpy as np

sys.path.insert(0, "/opt/trn_rl_repo")

import ml_dtypes  # noqa: E402
import concourse.bass as bass  # noqa: E402
import concourse.mybir as mybir  # noqa: E402
import concourse.tile as tile  # noqa: E402
from concourse import bacc  # noqa: E402
from concourse.bass_utils import run_bass_kernel_spmd  # noqa: E402
from concourse.masks import make_identity  # noqa: E402

BF16 = mybir.dt.bfloat16
F32 = mybir.dt.float32
AF = mybir.ActivationFunctionType

NCORES = 8
B, L, E, H = 2, 2048, 1024, 16
S = L
D = E // H            # 64 head dim
R = B * L             # 4096 total rows
HC = H // NCORES      # 2 heads per core
EC = HC * D           # 128 channel slice per core
KT = E // 128         # 8 contraction tiles
NT = L // 512         # 4 512-wide row tiles per batch
NP = L // 1024        # 2 1024-wide projection pairs per batch
ST = S // 128         # 16 key tiles per batch
STN = ST // NT        # 4 key tiles per 512-row block
DP1 = D + 1           # 65: head dim + ones column


def build_nc():
    nc = bacc.Bacc("TRN2", target_bir_lowering=False, num_devices=NCORES)

    qT = nc.declare_dram_parameter("qT", [128, B, 2, KT, 1024], BF16,
                                   isOutput=False)
    kT = nc.declare_dram_parameter("kT", [128, B, 2, KT, 1024], BF16,
                                   isOutput=False)
    vT = nc.declare_dram_parameter("vT", [128, B, 2, KT, 1024], BF16,
                                   isOutput=False)
    wq = nc.declare_dram_parameter("wq", [128, KT * EC], BF16, isOutput=False)
    wk = nc.declare_dram_parameter("wk", [128, KT * EC], BF16, isOutput=False)
    wv = nc.declare_dram_parameter("wv", [128, KT * EC], BF16, isOutput=False)
    wo = nc.declare_dram_parameter("wo", [128, E], BF16, isOutput=False)
    bq = nc.declare_dram_parameter("bq", [EC, 1], F32, isOutput=False)
    bk = nc.declare_dram_parameter("bk", [EC, 1], F32, isOutput=False)
    outTp = nc.declare_dram_parameter("outTp", [E, R], BF16, isOutput=True)

    with tile.TileContext(nc) as tc, ExitStack() as ctx:
        consts = ctx.enter_context(tc.tile_pool(name="consts", bufs=1))
        xt_pool = ctx.enter_context(tc.tile_pool(name="xt", bufs=1))
        vpt_pool = ctx.enter_context(tc.tile_pool(name="vpt", bufs=2))
        exp_pool = ctx.enter_context(tc.tile_pool(name="expp", bufs=8))
        otr_pool = ctx.enter_context(tc.tile_pool(name="otr", bufs=4))
        ot_pool = ctx.enter_context(tc.tile_pool(name="otp", bufs=2))
        ov_pool = ctx.enter_context(tc.tile_pool(name="ovp", bufs=3))
        rc_pool = ctx.enter_context(tc.tile_pool(name="rcp", bufs=4))
        rcb_pool = ctx.enter_context(tc.tile_pool(name="rcbp", bufs=4))
        # PSUM: sc 2x[128,1024] (4 banks) + pv 2 + proj chain 1 + outproj 1
        psum_sc = ctx.enter_context(tc.tile_pool(name="psc", bufs=2, space="PSUM"))
        psum_pv = ctx.enter_context(tc.tile_pool(name="ppv", bufs=2, space="PSUM"))
        psum_pp = ctx.enter_context(tc.tile_pool(name="ppp", bufs=1, space="PSUM"))
        psum_oj = ctx.enter_context(tc.tile_pool(name="poj", bufs=1, space="PSUM"))

        # ---- weights + biases on the sync ring (small, land ~3us)
        wq_sb = consts.tile([128, KT, EC], BF16, tag="wq")
        wk_sb = consts.tile([128, KT, EC], BF16, tag="wk")
        wv_sb = consts.tile([128, KT, EC], BF16, tag="wv")
        wo_sb = consts.tile([128, KT, EC], BF16, tag="wo")
        for w_sb, w in ((wk_sb, wk), (wq_sb, wq), (wv_sb, wv)):
            nc.sync.dma_start(w_sb[:], w[:].rearrange("p (ko m) -> p ko m", m=EC))
        nc.sync.dma_start(wo_sb[:], wo[:].rearrange("p (m o) -> p m o", o=EC))
        bq_sb = consts.tile([EC, 1], F32, tag="bq")
        bk_sb = consts.tile([EC, 1], F32, tag="bk")
        nc.sync.dma_start(bq_sb[:], bq[:])
        nc.sync.dma_start(bk_sb[:], bk[:])
        ident = consts.tile([128, 128], BF16, tag="ident")
        make_identity(nc, ident[:])

        # ---- staged activations: one [128, KT, L] buffer per tensor,
        # shared across batches (batch 1 overwrites once batch 0 is
        # consumed); filled in 512-row chunks on the gpsimd ring.
        staged = {}
        for name in ("k", "v", "q"):
            staged[name] = xt_pool.tile([128, KT, L], BF16, tag=f"xt{name}",
                                        name=f"xt{name}")
        xsrc = {"k": kT, "v": vT, "q": qT}

        def stage_chunk(name, b, c0, c1):
            """rows [512*c0, 512*c1) of batch b for tensor `name`."""
            for h in range(2):
                r0 = max(c0 * 512, h * 1024)
                r1 = min(c1 * 512, (h + 1) * 1024)
                if r0 >= r1:
                    continue
                nc.gpsimd.dma_start(
                    staged[name][:, :, r0:r1],
                    xsrc[name][:, b, h, :, r0 - h * 1024:r1 - h * 1024],
                )

        # projected activations (persistent, per batch)
        qpT = [[consts.tile([128, 1024], BF16, tag=f"qpT{b}_{p}",
                            name=f"qpT{b}_{p}") for p in range(NP)]
               for b in range(B)]
        kpT = [[consts.tile([128, 1024], BF16, tag=f"kpT{b}_{p}",
                            name=f"kpT{b}_{p}") for p in range(NP)]
               for b in range(B)]
        vp = [[consts.tile([128, STN, 2 * DP1], BF16, tag=f"vp{b}_{n}",
                           name=f"vp{b}_{n}")
               for n in range(NT)] for b in range(B)]
        for b in range(B):
            for n in range(NT):
                nc.vector.memset(vp[b][n][:, :, D], 1.0)
                nc.vector.memset(vp[b][n][:, :, 2 * D + 1], 1.0)

        # ---- HAM warmup: ~2.5us of real matmul activity during the first
        # DMA wait so projections run at 2.4GHz, not the cold 1.2GHz.
        # (transpose-mode does not count as PE-busy for HAM.)
        for _ in range(22):
            wps = psum_pp.tile([128, 128], F32, tag="pp", name="warm")
            nc.tensor.matmul(wps[:], lhsT=ident[:], rhs=ident[:],
                             start=True, stop=True)

        # ---------- filler units ----------
        # A unit is a list of (pe_cost, closure) ops.  Units are kept in a
        # FIFO; ops are popped a few per st (budget), gated on a DMA-
        # readiness st (gate) and force-drained at the start of the block
        # that consumes their output (need) so a consumer is never emitted
        # before its producer (deadlock-proof).

        def proj_chain_ops(b, name, p, hf):
            """8 matmuls + bias epilogue producing kpT/qpT[b][p] half hf."""
            w_sb, bias, dst = {
                "k": (wk_sb, bk_sb, kpT),
                "q": (wq_sb, bq_sb, qpT),
            }[name]
            box = {}
            ops = []
            for kt in range(KT):
                def mm(kt=kt):
                    if kt == 0:
                        box["ps"] = psum_pp.tile([128, 512], F32, tag="pp",
                                                 name="prch")
                    nc.tensor.matmul(
                        box["ps"][:], lhsT=w_sb[:, kt, :],
                        rhs=staged[name][:, kt,
                                         p * 1024 + hf * 512:
                                         p * 1024 + (hf + 1) * 512],
                        start=(kt == 0), stop=(kt == KT - 1),
                    )
                ops.append((1.0, mm))

            def epi():
                nc.vector.tensor_tensor(
                    dst[b][p][:, hf * 512:(hf + 1) * 512], box["ps"][:],
                    bias[:].to_broadcast((EC, 512)), mybir.AluOpType.add,
                )
            ops.append((0.1, epi))
            return ops

        def vproj_chain_ops(b, p, hf):
            """8 matmuls + cast + 4 transpose/copy ops into vp[b][2p+hf]."""
            n = p * 2 + hf
            box = {}
            ops = []
            for kt in range(KT):
                def mm(kt=kt):
                    if kt == 0:
                        box["ps"] = psum_pp.tile([128, 512], F32, tag="pp",
                                                 name="vch")
                    nc.tensor.matmul(
                        box["ps"][:], lhsT=wv_sb[:, kt, :],
                        rhs=staged["v"][:, kt,
                                        p * 1024 + hf * 512:
                                        p * 1024 + (hf + 1) * 512],
                        start=(kt == 0), stop=(kt == KT - 1),
                    )
                ops.append((1.0, mm))

            def cast():
                box["vpt"] = vpt_pool.tile([128, 512], BF16, tag="vpt", name="vptt")
                nc.vector.tensor_copy(box["vpt"][:], box["ps"][:])
            ops.append((0.1, cast))
            for sl in range(STN):
                def tp(sl=sl):
                    trp = psum_pp.tile([128, 128], BF16, tag="pp", name="vtp")
                    nc.tensor.transpose(
                        trp[:], box["vpt"][:, sl * 128:(sl + 1) * 128], ident[:]
                    )
                    nc.vector.tensor_copy(vp[b][n][:, sl, 0:D], trp[:, 0:D])
                    nc.vector.tensor_copy(vp[b][n][:, sl, DP1:DP1 + D],
                                          trp[:, D:2 * D])
                ops.append((0.7, tp))
            return ops

        fillers = []  # FIFO of {gate, need, ops: [(cost, op), ...]}

        def add_unit(gate, need, ops):
            fillers.append({"gate": gate, "need": need, "ops": list(ops)})

        ojq = []  # [(gate, op)] out-projection units, 1 popped per st

        def oj_unit(ot, rowbase, m, pool=None):
            def op():
                pt = (pool or psum_oj).tile([128, 512], F32,
                                            tag=("pp" if pool else "oj"),
                                            name="ojp")
                nc.tensor.matmul(pt[:], lhsT=wo_sb[:, m, :], rhs=ot[:],
                                 start=True, stop=True)
                ov = ov_pool.tile([128, 512], BF16, tag="ov", name="ovt")
                nc.vector.tensor_copy(ov[:], pt[:])
                nc.sync.dma_start(
                    outTp[m * 128:(m + 1) * 128, rowbase:rowbase + 512], ov[:]
                )
            return op

        def pop_fillers(g, budget=2.1):
            if ojq and g >= ojq[0][0]:
                ojq.pop(0)[1]()
                budget -= 1.0
            spent = 0.0
            while fillers and spent < budget:
                u = fillers[0]
                if g < u["gate"]:
                    break
                cost, op = u["ops"].pop(0)
                op()
                spent += cost
                if not u["ops"]:
                    fillers.pop(0)

        def force_units(max_need):
            """Emit every queued unit needed by st <= max_need (and, by
            FIFO, everything ahead of it)."""
            last = -1
            for i, u in enumerate(fillers):
                if u["need"] <= max_need:
                    last = i
            for u in fillers[:last + 1]:
                for _, op in u["ops"]:
                    op()
            del fillers[:last + 1]

        # ---------- core attention ops ----------

        def scores_exp(b, lt, st):
            ps = psum_sc.tile([128, 1024], F32, tag="sc", name="psc")
            for h in range(HC):
                nc.tensor.matmul(
                    ps[:, h * 512:(h + 1) * 512],
                    lhsT=kpT[b][st // 8][h * D:(h + 1) * D,
                                         (st % 8) * 128:(st % 8 + 1) * 128],
                    rhs=qpT[b][lt // 2][h * D:(h + 1) * D,
                                        (lt % 2) * 512:(lt % 2) * 512 + 512],
                    start=True, stop=True,
                    tile_position=(h * D, 0),
                )
            ex = exp_pool.tile([128, 1024], BF16, tag="exp", name="ext")
            nc.scalar.activation(ex[:], ps[:], AF.Exp)
            return ex

        def pv(b, po, st, ex, first, last):
            for h in range(HC):
                nc.tensor.matmul(
                    po[h][0:DP1, :],
                    lhsT=vp[b][st // STN][:, st % STN, h * DP1:(h + 1) * DP1],
                    rhs=ex[:, h * 512:(h + 1) * 512],
                    start=first, stop=last,
                )

        def norm_and_queue_oj(b, lt, po, g_end, tail=False):
            """Copy PSUM->SBUF (frees PV banks fast), normalize, queue the
            out-projection units (gated 3 sts later so the ot multiply has
            landed before the first oj matmul reaches the PE)."""
            otrs, rcs = [], []
            for h in range(HC):
                otr = otr_pool.tile([D, 512], F32, tag="otr", name="otrt")
                nc.vector.tensor_copy(otr[:], po[h][0:D, :])
                sm = rc_pool.tile([1, 512], F32, tag="sm", name="smt")
                nc.vector.tensor_copy(sm[:], po[h][D:DP1, :])
                otrs.append(otr)
                rcs.append(sm)
            for h in range(HC):
                rc1 = rc_pool.tile([1, 512], F32, tag="rc1", name="rc1t")
                nc.vector.reciprocal_approx_fast(rc1[:], rcs[h][:])
                rcs[h] = rc1
            rcbs = []
            for h in range(HC):
                rcb = rcb_pool.tile([D, 512], F32, tag="rcb", name="rcbt")
                nc.gpsimd.partition_broadcast(rcb[:], rcs[h][:])
                rcbs.append(rcb)
            ot = ot_pool.tile([128, 512], BF16, tag="ot", name="ott")
            for h in range(HC):
                nc.vector.tensor_tensor(
                    ot[h * D:(h + 1) * D, :], otrs[h][:], rcbs[h][:],
                    mybir.AluOpType.mult,
                )
            rowbase = b * L + lt * 512
            for m in range(KT):
                pool = psum_pp if (tail and m % 2 == 1) else None
                ojq.append((g_end + 3, oj_unit(ot, rowbase, m, pool=pool)))

        # ---------- staging schedule (gpsimd ring, need-order) ----------
        stage_chunk("k", 0, 0, 1)
        stage_chunk("q", 0, 0, 1)
        stage_chunk("v", 0, 0, 1)
        dma_plan = {
            1: ("k", 0, 1, 2), 3: ("k", 0, 2, 3), 5: ("v", 0, 1, 2),
            7: ("k", 0, 3, 4), 9: ("v", 0, 2, 3), 11: ("v", 0, 3, 4),
            13: ("q", 0, 1, 2), 15: ("q", 0, 2, 4),
            16: ("k", 1, 0, 2), 19: ("k", 1, 2, 4),
            23: ("v", 1, 0, 2), 27: ("q", 1, 0, 2),
            31: ("v", 1, 2, 4), 35: ("q", 1, 2, 4),
        }

        # ---------- pre-loop projections (block (0,0) first sts) --------
        for _, op in proj_chain_ops(0, "k", 0, 0):
            op()
        for _, op in proj_chain_ops(0, "q", 0, 0):
            op()

        # ---------- filler unit queue, ordered by need (FIFO) -----------
        # gate = earliest st the staging DMA has landed (pop paced);
        # need = first global st whose scores/PV consumes the output
        # (lookahead-forced so a consumer is never emitted first).
        add_unit(23, 32, proj_chain_ops(0, "q", 1, 0))   # blocks (0,2)+
        add_unit(25, 40, proj_chain_ops(0, "q", 1, 1))
        add_unit(28, 64, proj_chain_ops(1, "k", 0, 0))
        add_unit(39, 64, vproj_chain_ops(1, 0, 0))
        add_unit(44, 64, proj_chain_ops(1, "q", 0, 0))
        add_unit(29, 68, proj_chain_ops(1, "k", 0, 1))
        add_unit(40, 68, vproj_chain_ops(1, 0, 1))
        add_unit(34, 72, proj_chain_ops(1, "k", 1, 0))
        add_unit(49, 72, vproj_chain_ops(1, 1, 0))
        add_unit(35, 76, proj_chain_ops(1, "k", 1, 1))
        add_unit(50, 76, vproj_chain_ops(1, 1, 1))
        add_unit(45, 80, proj_chain_ops(1, "q", 0, 1))
        add_unit(54, 96, proj_chain_ops(1, "q", 1, 0))
        add_unit(55, 112, proj_chain_ops(1, "q", 1, 1))

        # ---------- block (0,0): hand-scheduled (DMA-paced) -------------
        # Interleaves the remaining batch-0 projections between the scores
        # (chains sequential: they share the single pp PSUM bank) with PV
        # emission deferred until each vp quarter's transposes are out.
        vch = {n: vproj_chain_ops(0, n // 2, n % 2) for n in range(NT)}
        kch = {1: proj_chain_ops(0, "k", 0, 1),
               2: proj_chain_ops(0, "k", 1, 0),
               3: proj_chain_ops(0, "k", 1, 1)}
        qch1 = proj_chain_ops(0, "q", 0, 1)
        # per-st: (list of (chain, lo, hi) to emit, PV sts to emit after)
        b0_plan = {
            1: ([(kch[1], 0, 4)], []),
            2: ([(kch[1], 4, 9)], []),
            4: ([(vch[0], 0, 5)], []),
            5: ([(vch[0], 5, 9)], []),
            6: ([(vch[0], 9, 13)], [0, 1, 2, 3]),
            7: ([(kch[2], 0, 9)], []),
            8: ([(vch[1], 0, 5)], []),
            9: ([(vch[1], 5, 9)], []),
            10: ([(vch[1], 9, 13)], [4, 5]),
            11: ([(kch[3], 0, 9)], [6, 7]),
            12: ([(vch[2], 0, 7)], []),
            13: ([(vch[2], 7, 13)], [8, 9]),
            14: ([(vch[3], 0, 8)], [10, 11]),
            15: ([(vch[3], 8, 13)], [12, 13, 14, 15]),
        }
        po = [psum_pv.tile([128, 512], F32, tag="pv", name=f"po{h}")
              for h in range(HC)]
        exs = {}
        for st in range(ST):
            if st in dma_plan:
                stage_chunk(*dma_plan[st])
            exs[st] = scores_exp(0, 0, st)
            pre, pvs = b0_plan.get(st, ([], []))
            for ch, o0, o1 in pre:
                for _, op in ch[o0:o1]:
                    op()
            for p_st in pvs:
                pv(0, po, p_st, exs.pop(p_st), first=(p_st == 0),
                   last=(p_st == ST - 1))
            if st == 15:
                for _, op in qch1:  # block (0,1) queries
                    op()
        norm_and_queue_oj(0, 0, po, 15)

        # ---------- blocks 1..7: generic scalar-paced loop --------------
        for blk in range(1, 8):
            b, lt = blk // 4, blk % 4
            po = [psum_pv.tile([128, 512], F32, tag="pv", name=f"po{h}")
                  for h in range(HC)]
            for st in range(ST):
                g = blk * 16 + st
                if g in dma_plan:
                    stage_chunk(*dma_plan[g])
                ex = scores_exp(b, lt, st)
                pv(b, po, st, ex, first=(st == 0), last=(st == ST - 1))
                pop_fillers(g)
                force_units(g + 3)
            norm_and_queue_oj(b, lt, po, blk * 16 + 15, tail=(blk == 7))

        # ---------- tail flush ----------
        while fillers:
            u = fillers.pop(0)
            for _, op in u["ops"]:
                op()
        while ojq:
            ojq.pop(0)[1]()

    nc.compile()
    return nc


_NC_CACHE = {}


def _get_nc():
    if "nc" not in _NC_CACHE:
        _NC_CACHE["nc"] = build_nc()
    return _NC_CACHE["nc"]


def _prearrange(w):
    # [E, EC] -> [128, KT*EC] partition-major so the device DMA is contiguous
    bf = ml_dtypes.bfloat16
    return np.ascontiguousarray(
        w.reshape(KT, 128, EC).transpose(1, 0, 2).reshape(128, KT * EC)
    ).astype(bf)


def kernel(q, k, v, Wq, bq, Wk, bk, Wv, bv, Wo, bo, _trace=False, _tmpdir=None):
    bf = ml_dtypes.bfloat16
    scale = np.float32(1.0 / np.sqrt(D))  # 0.125, exact

    def _stage_x(x):
        # [B, L, E] -> [128, B, 2, KT, 1024]: partition-major staging with
        # each L-half contiguous per partition (long DMA lines, few issues)
        xt = np.asarray(x, np.float32).reshape(B, 2, 1024, KT, 128)
        return np.ascontiguousarray(xt.transpose(4, 0, 1, 3, 2)).astype(bf)

    qTh = _stage_x(q)
    kTh = _stage_x(k)
    vTh = _stage_x(v)
    Wq = np.asarray(Wq, np.float32)
    Wk = np.asarray(Wk, np.float32)
    Wv = np.asarray(Wv, np.float32)
    Wo = np.asarray(Wo, np.float32)

    in_maps = []
    for c in range(NCORES):
        sl = slice(c * EC, (c + 1) * EC)
        in_maps.append({
            "qT": qTh,
            "kT": kTh,
            "vT": vTh,
            "wq": _prearrange(Wq[:, sl] * scale),
            "wk": _prearrange(Wk[:, sl]),
            "wv": _prearrange(Wv[:, sl]),
            "wo": np.ascontiguousarray(Wo[sl, :]).astype(bf),
            "bq": (np.asarray(bq, np.float32)[sl] * scale).reshape(EC, 1).copy(),
            "bk": np.asarray(bk, np.float32)[sl].reshape(EC, 1).copy(),
        })

    nc = _get_nc()
    res = run_bass_kernel_spmd(
        nc, in_maps, list(range(NCORES)), trace=_trace, tmpdir=_tmpdir
    )
    # sum the per-core partial outputs (the all-reduce of the TP sharding)
    acc = np.zeros((E, R), np.float32)
    for c in range(NCORES):
        acc += np.asarray(res.results[c]["outTp"], np.float32)
    out = np.ascontiguousarray(acc.T)  # [R, E]
    # bv passes through attention unchanged (softmax rows sum to 1):
    # out += bv @ Wo + bo
    host_bias = (
        np.asarray(bv, np.float64) @ np.asarray(Wo, np.float64)
        + np.asarray(bo, np.float64)
    ).astype(np.float32)
    out += host_bias[None, :]
    if _trace:
        return out.reshape(B, L, E), res
    return out.reshape(B, L, E)
